# revision 3
# baseline (speedup 1.0000x reference)
"""Trainium2 Bass kernel for nn_MultiHeadModel (moe_routing).

Reference computation:
    route  = argmax(x @ W_lab + b_lab, -1)            # [N]
    z      = x @ W_enc + b_enc                        # [N, 64]
    heads  = einsum('nd,ids->nis', z, W_clf) + b_clf  # [N, 8, 4]
    out    = (heads * onehot(route)).reshape(N, 32)

Key algebraic fold (all maps linear): heads = x @ W_eff + b_eff with
    W_eff = W_enc @ W_clf_flat,  W_clf_flat[d, i*4+s] = W_clf[i, d, s]
    b_eff = b_enc @ W_clf_flat + b_clf.ravel()
so the whole model is one matmul against W_big = [W_lab | W_eff] : [128, 40]
followed by an argmax-mask over the first 8 columns.

Device pipeline (pure data parallel over 8 cores, 65536 rows each):
  - DMA macro-tiles of 2048 tokens: SBUF partition p holds tokens 16p..16p+15
    (keeps every DMA descriptor >= 2KB contiguous for loads AND stores).
  - PE transpose (via identity) each [128 tok, 128 d] sub-tile -> xT in PSUM.
  - ACT copies xT PSUM->SBUF (4 sub-tiles per copy).
  - PE matmul: lhsT = xT sub-tile (stationary), rhs = W_big [128, 40] ->
    token-major PSUM [128 tok, 40] (cols 0:8 logits, 8:40 heads).
  - DVE: segmented reduce_max over logits, is_equal -> one-hot mask,
    masked multiply of heads -> SBUF output tile.
  - DMA store [128, 16*32] with 2KB/partition contiguous descriptors.
"""

import sys

if "/opt/trn_rl_repo" not in sys.path:
    sys.path.insert(0, "/opt/trn_rl_repo")

import numpy as np

N_TOTAL = 524288
N_CORES = 8
N_PER_CORE = N_TOTAL // N_CORES  # 65536
D_IN = 128
Y_DIM = 8
S_DIM = 4
D_ENC = 64
W_COLS = Y_DIM + Y_DIM * S_DIM  # 40
OUT_COLS = Y_DIM * S_DIM  # 32

G = 16                    # tokens per partition per macro-tile
MACRO = 128 * G           # 2048 tokens per macro-tile
N_MACROS = N_PER_CORE // MACRO  # 32

_CACHE = {}

# test.py can read this after calling kernel() to get profile info
LAST_RESULTS = None


def _build(with_bias: bool):
    import concourse.bass as bass
    import concourse.bacc as bacc
    import concourse.mybir as mybir
    import concourse.tile as tile
    from concourse import masks

    f32 = mybir.dt.float32
    nc = bacc.Bacc("TRN2", target_bir_lowering=False)

    x_d = nc.dram_tensor("x", [N_PER_CORE, D_IN], f32, kind="ExternalInput")
    w_d = nc.dram_tensor("w_big", [D_IN, W_COLS], f32, kind="ExternalInput")
    if with_bias:
        b_d = nc.dram_tensor("b_big", [1, W_COLS], f32, kind="ExternalInput")
    out_d = nc.dram_tensor("out", [N_PER_CORE, OUT_COLS], f32, kind="ExternalOutput")

    with tile.TileContext(nc) as tc:
        with (
            tc.tile_pool(name="const", bufs=1) as const_pool,
            tc.tile_pool(name="xin", bufs=3) as x_pool,
            tc.tile_pool(name="xts", bufs=4) as xts_pool,
            tc.tile_pool(name="outs", bufs=3) as out_pool,
            tc.tile_pool(name="small", bufs=4) as small_pool,
            tc.tile_pool(name="xtp", bufs=3, space=bass.MemorySpace.PSUM) as xtp_pool,
            tc.tile_pool(name="bigp", bufs=3, space=bass.MemorySpace.PSUM) as bigp_pool,
        ):
            ident = const_pool.tile([128, 128], f32)
            masks.make_identity(nc, ident[:])
            w_sb = const_pool.tile([D_IN, W_COLS], f32)
            nc.sync.dma_start(w_sb[:], w_d[:])

            if with_bias:
                # replicate bias across partitions: ones[1,128].T @ b[1,40]
                ones_sb = const_pool.tile([1, 128], f32)
                nc.gpsimd.memset(ones_sb[:], 1.0)
                b_row = const_pool.tile([1, W_COLS], f32)
                nc.sync.dma_start(b_row[:], b_d[:])
                with tc.tile_pool(
                    name="biasp", bufs=1, space=bass.MemorySpace.PSUM
                ) as biasp_pool:
                    bias_ps = biasp_pool.tile([128, W_COLS], f32)
                    nc.tensor.matmul(bias_ps[:], ones_sb[:], b_row[:])
                    bias_sb = const_pool.tile([128, W_COLS], f32)
                    nc.scalar.copy(bias_sb[:], bias_ps[:])

            for m in range(N_MACROS):
                r0 = m * MACRO
                x_sb = x_pool.tile([128, G * D_IN], f32)
                nc.sync.dma_start(
                    x_sb[:],
                    x_d[r0 : r0 + MACRO, :].rearrange("(p g) d -> p (g d)", p=128),
                )
                out_sb = out_pool.tile([128, G, OUT_COLS], f32)

                for half in range(2):
                    big_ps = bigp_pool.tile([128, G // 2, W_COLS], f32)
                    for h in range(2):
                        xt_ps = xtp_pool.tile([128, 512], f32)
                        for q in range(4):
                            g = half * 8 + h * 4 + q
                            nc.tensor.transpose(
                                xt_ps[:, q * 128 : (q + 1) * 128],
                                x_sb[:, g * D_IN : (g + 1) * D_IN],
                                ident[:],
                            )
                        xt_sb = xts_pool.tile([128, 512], f32)
                        nc.scalar.copy(xt_sb[:], xt_ps[:])
                        for q in range(4):
                            nc.tensor.matmul(
                                big_ps[:, h * 4 + q, :],
                                xt_sb[:, q * 128 : (q + 1) * 128],
                                w_sb[:],
                            )

                    if with_bias:
                        nc.vector.tensor_tensor(
                            big_ps[:],
                            big_ps[:],
                            bias_sb[:][:, None, :].broadcast_to(
                                [128, G // 2, W_COLS]
                            ),
                            mybir.AluOpType.add,
                        )

                    maxl = small_pool.tile([128, G // 2], f32)
                    nc.vector.tensor_reduce(
                        maxl[:],
                        big_ps[:, :, 0:Y_DIM],
                        axis=mybir.AxisListType.X,
                        op=mybir.AluOpType.max,
                    )
                    mask = small_pool.tile([128, G // 2, Y_DIM], f32)
                    nc.vector.tensor_tensor(
                        mask[:],
                        big_ps[:, :, 0:Y_DIM],
                        maxl[:][:, :, None].broadcast_to([128, G // 2, Y_DIM]),
                        mybir.AluOpType.is_equal,
                    )
                    nc.vector.tensor_tensor(
                        out_sb[:, half * 8 : half * 8 + G // 2, :].rearrange(
                            "p g (i s) -> p g i s", s=S_DIM
                        ),
                        big_ps[:, :, Y_DIM:W_COLS].rearrange(
                            "p g (i s) -> p g i s", s=S_DIM
                        ),
                        mask[:][:, :, :, None].broadcast_to(
                            [128, G // 2, Y_DIM, S_DIM]
                        ),
                        mybir.AluOpType.mult,
                    )

                nc.sync.dma_start(
                    out_d[r0 : r0 + MACRO, :].rearrange("(p g) j -> p (g j)", p=128),
                    out_sb[:],
                )

    nc.compile()
    return nc


def _get_nc(with_bias: bool):
    key = ("nc", with_bias)
    if key not in _CACHE:
        _CACHE[key] = _build(with_bias)
    return _CACHE[key]


def kernel(x, W_lab, b_lab, W_enc, b_enc, W_clf, b_clf):
    global LAST_RESULTS
    from concourse.bass_utils import run_bass_kernel_spmd

    x = np.ascontiguousarray(np.asarray(x, dtype=np.float32))
    W_lab = np.asarray(W_lab, dtype=np.float32)
    b_lab = np.asarray(b_lab, dtype=np.float32)
    W_enc = np.asarray(W_enc, dtype=np.float32)
    b_enc = np.asarray(b_enc, dtype=np.float32)
    W_clf = np.asarray(W_clf, dtype=np.float32)
    b_clf = np.asarray(b_clf, dtype=np.float32)

    # Fold encoder + classifier into one [128, 32] map (all linear).
    w_clf_flat = np.transpose(W_clf, (1, 0, 2)).reshape(D_ENC, OUT_COLS)
    w_eff = (W_enc.astype(np.float64) @ w_clf_flat.astype(np.float64)).astype(
        np.float32
    )
    b_eff = (
        b_enc.astype(np.float64) @ w_clf_flat.astype(np.float64)
        + b_clf.reshape(OUT_COLS).astype(np.float64)
    ).astype(np.float32)
    w_big = np.ascontiguousarray(
        np.concatenate([W_lab, w_eff], axis=1), dtype=np.float32
    )  # [128, 40]
    b_big = np.concatenate([b_lab, b_eff]).astype(np.float32)  # [40]

    with_bias = bool(np.any(b_big != 0.0))
    nc = _get_nc(with_bias)

    in_maps = []
    for i in range(N_CORES):
        m = {
            "x": x[i * N_PER_CORE : (i + 1) * N_PER_CORE],
            "w_big": w_big,
        }
        if with_bias:
            m["b_big"] = b_big.reshape(1, W_COLS)
        in_maps.append(m)

    res = run_bass_kernel_spmd(nc, in_maps, list(range(N_CORES)))
    LAST_RESULTS = res
    out = np.concatenate(
        [res.results[i]["out"] for i in range(N_CORES)], axis=0
    ).astype(np.float32)
    return out
